# revision 36
# baseline (speedup 1.0000x reference)
"""AttentionPool Trainium2 kernel (v5).

Computes, for x (B,T,m), W1 (m,m), W2 (m,m), vm (1,m):
    h      = tanh(x @ W1 + vm @ W2)          (B,T,m)
    scores = h @ vm[0]                       (B,T,1)
    w      = softmax(scores, axis=T)
    out    = sum(x * w, axis=T, keepdims)    (B,1,m)

Sharding: data-parallel over B across 8 NeuronCores (2 examples per core);
W1/W2/vm replicated.  No max-subtraction needed: |scores| <= ||vm||_1.

v5 (all-bf16, engine-balanced; v4's fp8 DoubleRow h-matmul was 68.6us but
missed the accuracy gate at 2.6e-2):
  - h-matmul: 4x 1024-col bf16 matmuls per chunk (per-nh PSUM tile
    [128,1024], 3-deep ring = 6 banks) instead of 8x 512-col.
  - tanh: 2x 1024-col ACT calls per chunk (bias folds per-nh) instead of
    4x 512-col: ~ -9us ACT.
  - scores: DVE pre-folds the two n-halves, g2 = h0*vm0 + h1*vm1
    (tensor_scalar_mul 4x-mode + scalar_tensor_tensor), with g2 written
    r-major via a transposed access pattern so the 8 score matmuls per
    chunk use CONTIGUOUS 128-col stationaries (FWL, ~53ns LDWEIGHTS) and
    a constant ones-column moving operand.  Halves the PE score cost vs
    16 strided LDW-bound pairs.
  - pooling: x-block stationaries (FWL) with 1-col e moving operand —
    rides the LDWEIGHTS port under the h-matmul stream.
  - DMA: 4096-t granules (xts descriptors 8KB), xts prefetch prioritized
    over xin (pool trails by 2 chunks).
"""

import numpy as np
import ml_dtypes

import concourse.bass as bass
import concourse.tile as tile
from concourse import bacc, bass_isa, mybir
from concourse.bass_utils import run_bass_kernel_spmd

FP32 = mybir.dt.float32
BF16 = mybir.dt.bfloat16
AF = mybir.ActivationFunctionType
ALU = mybir.AluOpType

N_CORES = 8
B = 16
B_PER_CORE = B // N_CORES  # 2
T = 8192
M = 256
P = 128
CHUNK = 1024         # t rows per chunk
NT = CHUNK // P      # 8: t = c*1024 + tau*8 + r
NCHUNK = T // CHUNK  # 8 chunks per example
NE = NCHUNK * NT     # e columns per example (64)
GC = 4               # chunks per DMA granule
GRAN = GC * CHUNK    # 4096 t rows per granule
NGRAN = T // GRAN    # 2 granules per example


def _build_program() -> bass.Bass:
    nc = bacc.Bacc("TRN2", target_bir_lowering=False, debug=False)

    x = nc.dram_tensor("x", [B_PER_CORE, T, M], BF16, kind="ExternalInput")
    xT = nc.dram_tensor("xT", [B_PER_CORE, M, T], BF16, kind="ExternalInput")
    # W1b[p, mh*256+n] = W1[mh*128+p, n], host-cast bf16
    W1b = nc.dram_tensor("W1b", [P, 2 * M], BF16, kind="ExternalInput")
    # CV[:, 0:2] = vm transposed (vmt[p, i] = vm[i*128+p]);
    # CV[:, 2:4] = c = vm @ W2, same transposed layout.  Host-computed:
    # keeps the setup DMAs descriptor-friendly (the v5.2 on-chip variant
    # gated all PE work behind 256 four-byte DMA descriptors).
    CV = nc.dram_tensor("CV", [P, 4], FP32, kind="ExternalInput")
    out = nc.dram_tensor("out", [B_PER_CORE, M], FP32, kind="ExternalOutput")

    with tile.TileContext(nc) as tc:
        with (
            tc.tile_pool(name="setup", bufs=1) as setup,
            tc.tile_pool(name="xin", bufs=3) as xin_pool,
            tc.tile_pool(name="xts", bufs=4) as xts_pool,
            tc.tile_pool(name="hps", bufs=3, space="PSUM") as hps_pool,
            tc.tile_pool(name="hsb", bufs=3) as hsb_pool,
            tc.tile_pool(name="ggg", bufs=2) as g_pool,
            tc.tile_pool(name="sps", bufs=1, space="PSUM") as sps_pool,
            tc.tile_pool(name="acc", bufs=1, space="PSUM") as acc_pool,
            tc.tile_pool(name="eee", bufs=1) as e_pool,
            tc.tile_pool(name="fin", bufs=2) as fin_pool,
        ):
            # ---------------- setup ----------------
            NCT = B_PER_CORE * NCHUNK   # 16 global chunks
            NGT = B_PER_CORE * NGRAN    # 4 global granules
            xin_t = [None] * NCT
            xts_t = [None] * NCT
            hsb_t = [None] * NCT

            # chunk 0's xts is the very first dma_start: the descriptor
            # generator serializes dma_starts (~0.6us each), and this one
            # gates the first h-matmul.
            xts_c0 = xts_pool.tile([P, 2, CHUNK], BF16, tag="xts00")
            nc.sync.dma_start(
                out=xts_c0,
                in_=xT[0, :, 0:CHUNK].rearrange("(a p) t -> p a t", p=P),
            )
            xts_t[0] = xts_c0

            w1b = setup.tile([P, 2, M], BF16)
            nc.sync.dma_start(out=w1b, in_=W1b.rearrange("p (a n) -> p a n", a=2))

            cv = setup.tile([P, 4], FP32)
            nc.sync.dma_start(out=cv, in_=CV[:, :])
            vmt_f = cv[:, 0:2]
            c_sb = cv[:, 2:4]

            ones_col_b = setup.tile([P, 1], BF16)
            nc.vector.memset(ones_col_b, 1.0)

            # HAM warm-up: ~5us of dummy matmul activity bridging the gap
            # from the preamble (~6.5us) to the first x-chunk's arrival
            # (~12us) so the PE clock-gate (3.4us window) is open and stays
            # open when the real h-matmuls start.  Memset operands: no DMA
            # dependency.  512-col matmuls (~245ns each) keep the PE
            # continuously busy rather than idling between tiny ops.
            warm_w = setup.tile([P, P], BF16)
            nc.vector.memset(warm_w, 0.001)
            warm_rhs = setup.tile([P, 512], BF16)
            nc.vector.memset(warm_rhs, 0.001)
            warm_ps = sps_pool.tile([P, 512], FP32, tag="sps")
            for _ in range(20):
                nc.tensor.matmul(
                    warm_ps,
                    lhsT=warm_w,
                    rhs=warm_rhs,
                    start=True,
                    stop=True,
                )

            ones_col = setup.tile([P, 1], FP32)
            nc.vector.memset(ones_col, 1.0)
            ones_row = setup.tile([1, P], FP32)
            nc.vector.memset(ones_row, 1.0)

            # ---------------- main loop ----------------
            e_t = [None] * B_PER_CORE
            acc_t = [None] * B_PER_CORE

            def emit_xts(g, split=False):
                b, gb = divmod(g, NGRAN)
                if split:
                    for cc in range(GC):
                        if xts_t[g * GC + cc] is not None:
                            continue
                        xts_c = xts_pool.tile([P, 2, CHUNK], BF16, tag=f"xts0{cc}")
                        nc.sync.dma_start(
                            out=xts_c,
                            in_=xT[b, :, gb * GRAN + cc * CHUNK :
                                   gb * GRAN + (cc + 1) * CHUNK].rearrange(
                                "(a p) t -> p a t", p=P
                            ),
                        )
                        xts_t[g * GC + cc] = xts_c
                else:
                    xts = xts_pool.tile([P, 2, GRAN], BF16)
                    nc.sync.dma_start(
                        out=xts,
                        in_=xT[b, :, gb * GRAN : (gb + 1) * GRAN].rearrange(
                            "(a p) t -> p a t", p=P
                        ),
                    )
                    for cc in range(GC):
                        xts_t[g * GC + cc] = xts[:, :, cc * CHUNK : (cc + 1) * CHUNK]

            def emit_xin(g):
                b, gb = divmod(g, NGRAN)
                # xin[p, cc, r, m] = x[b, gb*4096 + cc*1024 + p*8 + r, m]
                xin = xin_pool.tile([P, GC, NT, M], BF16)
                nc.sync.dma_start(
                    out=xin,
                    in_=x[b, gb * GRAN : (gb + 1) * GRAN, :].rearrange(
                        "(cc p r) m -> p cc r m", p=P, r=NT
                    ),
                )
                for cc in range(GC):
                    xin_t[g * GC + cc] = xin[:, cc]

            g_t = [None] * NCT

            def emit_fold(ct):
                # DVE fold of the two n-halves: g = h0*vm0 + h1*vm1
                # (per-partition scalars; the partition sum happens on PE).
                # Emitted one iteration before its score matmuls so the
                # ~1.8us DVE chain overlaps a full chunk of PE work.
                # Dense writes only: a transposed out AP drops the DVE to
                # a ~5x slower path (measured).
                hsb = hsb_t[ct]
                g0 = g_pool.tile([P, P, NT], BF16, tag="g0")
                nc.vector.tensor_scalar_mul(g0, hsb[:, 0], vmt_f[:, 0:1])
                g = g_pool.tile([P, P, NT], BF16, tag="gg")
                nc.vector.scalar_tensor_tensor(
                    g, hsb[:, 1], vmt_f[:, 1:2], g0, ALU.mult, ALU.add
                )
                g_t[ct] = g
                hsb_t[ct] = None

            def emit_step(ct, sct, pct):
                """One pipeline step: h-matmuls for chunk ct interleaved
                with score matmuls for sct and pool matmuls for pct, so the
                512-col h streams cover the small matmuls' weight loads
                (one background weight slot => only loads issued during a
                long matmul are free)."""
                small = []
                if sct is not None:
                    b, c = divmod(sct, NCHUNK)
                    e_all = e_t[b]
                    g = g_t[sct]
                    sps = sps_pool.tile([P, NT], FP32, tag="sps")

                    def mk_sc(r):
                        def f():
                            # s[tau, r] = sum_p g[p, tau*8+r]
                            nc.tensor.matmul(
                                sps[:, r : r + 1],
                                lhsT=g[:, :, r],
                                rhs=ones_col_b,
                                start=True,
                                stop=True,
                            )
                        return f

                    small += [mk_sc(r) for r in range(NT)]
                if pct is not None:
                    bp, cp = divmod(pct, NCHUNK)
                    e_allp = e_t[bp]
                    acc_ps = acc_t[bp]
                    xin = xin_t[pct]

                    def mk_pool(r, mh):
                        def f():
                            # acc[q, mh] += sum_p x[t(p,r), mh*128+q]*e[t(p,r)]
                            nc.tensor.matmul(
                                acc_ps[:, mh : mh + 1],
                                lhsT=xin[:, r, mh * P : (mh + 1) * P],
                                rhs=e_allp[:, cp * NT + r : cp * NT + r + 1],
                                start=False,
                                stop=(cp == NCHUNK - 1 and r == NT - 1),
                                skip_group_check=True,
                            )
                        return f

                    small += [mk_pool(r, mh) for r in range(NT) for mh in range(2)]

                if ct is not None:
                    xts = xts_t[ct]
                    hsb = hsb_pool.tile([P, 2, P, NT], BF16)
                    hsb_t[ct] = hsb
                    si = 0
                    # h as 8x 512-col streams (512 = one PSUM bank: start
                    # clears has_written for the WHOLE bank, so start=True
                    # matmuls must cover full banks); 3 small matmuls after
                    # each h stream.  Finer splits measured slower.
                    for nh in range(2):
                        hps = hps_pool.tile([P, CHUNK], FP32)
                        for mh in range(2):
                            for hf in range(2):
                                nc.tensor.matmul(
                                    hps[:, hf * 512 : (hf + 1) * 512],
                                    lhsT=w1b[:, mh, nh * P : (nh + 1) * P],
                                    rhs=xts[:, mh, hf * 512 : (hf + 1) * 512],
                                    start=(mh == 0),
                                    stop=(mh == 1),
                                )
                                for _ in range(3):
                                    if si < len(small):
                                        small[si]()
                                        si += 1
                        nc.scalar.activation(
                            hsb[:, nh],
                            hps,
                            AF.Tanh,
                            bias=c_sb[:, nh : nh + 1],
                        )
                    while si < len(small):
                        small[si]()
                        si += 1
                else:
                    for f in small:
                        f()

                if sct is not None:
                    b, c = divmod(sct, NCHUNK)
                    nc.scalar.activation(
                        e_t[b][:, c * NT : (c + 1) * NT],
                        sps,
                        AF.Exp,
                    )
                if pct is not None:
                    xin_t[pct] = None

            def emit_finalize(b, use_gpsimd=False):
                e_all = e_t[b]
                acc_ps = acc_t[b]
                # Z = sum(e_all): free-dim reduce on DVE, then partition
                # reduce.  For example 0 (mid-pipeline) the partition reduce
                # runs on the idle GPSIMD so the in-order PE queue is not
                # blocked waiting on the DVE (which is busy with g-folds).
                z_red = fin_pool.tile([P, 1], FP32)
                nc.vector.reduce_sum(z_red, e_all, axis=mybir.AxisListType.X)
                rz = fin_pool.tile([P, 1], FP32, name=f"rz{b}")
                if use_gpsimd:
                    z_all = fin_pool.tile([P, 1], FP32, name="z_all")
                    nc.gpsimd.partition_all_reduce(
                        z_all, z_red, P, bass_isa.ReduceOp.add
                    )
                    nc.vector.reciprocal(rz, z_all)
                else:
                    z_ps = sps_pool.tile([1, 1], FP32, tag="sps")
                    nc.tensor.matmul(
                        z_ps, lhsT=z_red, rhs=ones_col, start=True, stop=True
                    )
                    z_sb = fin_pool.tile([1, 1], FP32)
                    nc.vector.tensor_copy(z_sb, z_ps)
                    # broadcast Z to all partitions, then reciprocal
                    zb_ps = sps_pool.tile([P, 1], FP32, tag="sps")
                    nc.tensor.matmul(
                        zb_ps, lhsT=ones_row, rhs=z_sb, start=True, stop=True
                    )
                    nc.vector.reciprocal(rz, zb_ps)
                # scale pooled sums by 1/Z; acc_ps is already m-partitioned
                outsb = fin_pool.tile([P, 2], FP32)
                nc.vector.tensor_scalar_mul(outsb, acc_ps, rz)
                nc.sync.dma_start(
                    out=out[b].rearrange("(a p) -> p a", p=P), in_=outsb
                )

            # both examples' accumulators share one PSUM bank ([P, 4]):
            # start=True would zero the whole bank row, wiping the sibling
            # chains; memset once and accumulate-only.
            acc_all = acc_pool.tile([P, 2 * B_PER_CORE], FP32, name="acc")
            nc.vector.memset(acc_all, 0.0)
            for b in range(B_PER_CORE):
                e_t[b] = e_pool.tile([P, NE], BF16, name=f"e_all{b}")
                acc_t[b] = acc_all[:, 2 * b : 2 * b + 2]

            # xts prefetch has priority over xin (pool trails h by 2 chunks)
            emit_xts(0, split=True)
            emit_xts(1)
            emit_xin(0)
            emit_xin(1)
            # scores lag h by 2 chunks, pool by 3: the per-chunk chain
            # (h-MM -> tanh -> DVE fold -> score-MM -> exp -> pool) is
            # ~4us, longer than one chunk of PE work, so a 1-chunk lag
            # stalls the PE waiting on the fold.
            for ct in range(NCT):
                # prefetch 2 granules ahead, issued early (ct%GC==2) so the
                # transfer completes well before the consuming chunk
                if ct % GC == 2 and ct // GC + 2 < NGT:
                    emit_xts(ct // GC + 2)
                    emit_xin(ct // GC + 2)
                if ct >= 1:
                    emit_fold(ct - 1)
                emit_step(
                    ct,
                    ct - 2 if ct >= 2 else None,
                    ct - 3 if ct >= 3 else None,
                )
                if ct == NCHUNK + 3:
                    emit_finalize(0, use_gpsimd=True)
            emit_fold(NCT - 1)
            emit_step(None, NCT - 2, NCT - 3)
            emit_step(None, NCT - 1, NCT - 2)
            emit_step(None, None, NCT - 1)
            emit_finalize(1, use_gpsimd=True)

    return nc


_PROGRAM_CACHE: list = []


def _get_program() -> bass.Bass:
    if not _PROGRAM_CACHE:
        nc = _build_program()
        nc.finalize()
        _PROGRAM_CACHE.append(nc)
    return _PROGRAM_CACHE[0]


def _make_in_maps(x, W1, W2, vm):
    xb = np.ascontiguousarray(x).astype(ml_dtypes.bfloat16)
    xbT = np.ascontiguousarray(xb.transpose(0, 2, 1))
    W1 = np.asarray(W1, dtype=np.float32)
    W2 = np.asarray(W2, dtype=np.float32)
    vm = np.asarray(vm, dtype=np.float32)
    # W1b[p, mh*256+n] = W1[mh*128+p, n]
    W1b = np.ascontiguousarray(
        W1.reshape(2, P, M).transpose(1, 0, 2).reshape(P, 2 * M)
    ).astype(ml_dtypes.bfloat16)
    c = (vm @ W2)[0]  # (256,)
    CV = np.ascontiguousarray(
        np.stack(
            [vm[0, :P], vm[0, P:], c[:P], c[P:]], axis=1
        ),
        dtype=np.float32,
    )
    return [
        {
            "x": xb[i * B_PER_CORE : (i + 1) * B_PER_CORE],
            "xT": xbT[i * B_PER_CORE : (i + 1) * B_PER_CORE],
            "W1b": W1b,
            "CV": CV,
        }
        for i in range(N_CORES)
    ]


def kernel(x, W1, W2, vm):
    nc = _get_program()
    in_maps = _make_in_maps(x, W1, W2, vm)
    res = run_bass_kernel_spmd(nc, in_maps, list(range(N_CORES)))
    out = np.concatenate([res.results[i]["out"] for i in range(N_CORES)], axis=0)
    return out.reshape(B, 1, M).astype(np.float32)


# revision 38
# speedup vs baseline: 1.0643x; 1.0643x over previous
"""AttentionPool Trainium2 kernel (v5).

Computes, for x (B,T,m), W1 (m,m), W2 (m,m), vm (1,m):
    h      = tanh(x @ W1 + vm @ W2)          (B,T,m)
    scores = h @ vm[0]                       (B,T,1)
    w      = softmax(scores, axis=T)
    out    = sum(x * w, axis=T, keepdims)    (B,1,m)

Sharding: data-parallel over B across 8 NeuronCores (2 examples per core);
W1/W2/vm replicated.  No max-subtraction needed: |scores| <= ||vm||_1.

v5 (all-bf16, engine-balanced; v4's fp8 DoubleRow h-matmul was 68.6us but
missed the accuracy gate at 2.6e-2):
  - h-matmul: 4x 1024-col bf16 matmuls per chunk (per-nh PSUM tile
    [128,1024], 3-deep ring = 6 banks) instead of 8x 512-col.
  - tanh: 2x 1024-col ACT calls per chunk (bias folds per-nh) instead of
    4x 512-col: ~ -9us ACT.
  - scores: DVE pre-folds the two n-halves, g2 = h0*vm0 + h1*vm1
    (tensor_scalar_mul 4x-mode + scalar_tensor_tensor), with g2 written
    r-major via a transposed access pattern so the 8 score matmuls per
    chunk use CONTIGUOUS 128-col stationaries (FWL, ~53ns LDWEIGHTS) and
    a constant ones-column moving operand.  Halves the PE score cost vs
    16 strided LDW-bound pairs.
  - pooling: x-block stationaries (FWL) with 1-col e moving operand —
    rides the LDWEIGHTS port under the h-matmul stream.
  - DMA: 4096-t granules (xts descriptors 8KB), xts prefetch prioritized
    over xin (pool trails by 2 chunks).
"""

import numpy as np
import ml_dtypes

import concourse.bass as bass
import concourse.tile as tile
from concourse import bacc, bass_isa, mybir
from concourse.bass_utils import run_bass_kernel_spmd

FP32 = mybir.dt.float32
BF16 = mybir.dt.bfloat16
AF = mybir.ActivationFunctionType
ALU = mybir.AluOpType

N_CORES = 8
B = 16
B_PER_CORE = B // N_CORES  # 2
T = 8192
M = 256
P = 128
CHUNK = 1024         # t rows per chunk
NT = CHUNK // P      # 8: t = c*1024 + tau*8 + r
NCHUNK = T // CHUNK  # 8 chunks per example
NE = NCHUNK * NT     # e columns per example (64)
GC = 4               # chunks per DMA granule
GRAN = GC * CHUNK    # 4096 t rows per granule
NGRAN = T // GRAN    # 2 granules per example


def _build_program() -> bass.Bass:
    nc = bacc.Bacc("TRN2", target_bir_lowering=False, debug=False)

    x = nc.dram_tensor("x", [B_PER_CORE, T, M], BF16, kind="ExternalInput")
    xT = nc.dram_tensor("xT", [B_PER_CORE, M, T], BF16, kind="ExternalInput")
    # W1b[p, mh*256+n] = W1[mh*128+p, n], host-cast bf16
    W1b = nc.dram_tensor("W1b", [P, 2 * M], BF16, kind="ExternalInput")
    # CV[:, 0:2] = vm transposed (vmt[p, i] = vm[i*128+p]);
    # CV[:, 2:4] = c = vm @ W2, same transposed layout.  Host-computed:
    # keeps the setup DMAs descriptor-friendly (the v5.2 on-chip variant
    # gated all PE work behind 256 four-byte DMA descriptors).
    CV = nc.dram_tensor("CV", [P, 4], FP32, kind="ExternalInput")
    out = nc.dram_tensor("out", [B_PER_CORE, M], FP32, kind="ExternalOutput")

    with tile.TileContext(nc) as tc:
        with (
            tc.tile_pool(name="setup", bufs=1) as setup,
            tc.tile_pool(name="xin", bufs=3) as xin_pool,
            tc.tile_pool(name="xts", bufs=4) as xts_pool,
            tc.tile_pool(name="hps", bufs=3, space="PSUM") as hps_pool,
            tc.tile_pool(name="hsb", bufs=3) as hsb_pool,
            tc.tile_pool(name="ggg", bufs=2) as g_pool,
            tc.tile_pool(name="sps", bufs=1, space="PSUM") as sps_pool,
            tc.tile_pool(name="acc", bufs=1, space="PSUM") as acc_pool,
            tc.tile_pool(name="eee", bufs=1) as e_pool,
            tc.tile_pool(name="fin", bufs=2) as fin_pool,
        ):
            # ---------------- setup ----------------
            NCT = B_PER_CORE * NCHUNK   # 16 global chunks
            NGT = B_PER_CORE * NGRAN    # 4 global granules
            xin_t = [None] * NCT
            xts_t = [None] * NCT
            hsb_t = [None] * NCT

            w1b = setup.tile([P, 2, M], BF16)
            nc.sync.dma_start(out=w1b, in_=W1b.rearrange("p (a n) -> p a n", a=2))

            cv = setup.tile([P, 4], FP32)
            nc.sync.dma_start(out=cv, in_=CV[:, :])
            vmt_f = cv[:, 0:2]
            c_sb = cv[:, 2:4]

            ones_col_b = setup.tile([P, 1], BF16)
            nc.vector.memset(ones_col_b, 1.0)

            # HAM warm-up: ~5us of dummy matmul activity bridging the gap
            # from the preamble (~6.5us) to the first x-chunk's arrival
            # (~12us) so the PE clock-gate (3.4us window) is open and stays
            # open when the real h-matmuls start.  Memset operands: no DMA
            # dependency.  512-col matmuls (~245ns each) keep the PE
            # continuously busy rather than idling between tiny ops.
            warm_w = setup.tile([P, P], BF16)
            nc.vector.memset(warm_w, 0.001)
            warm_rhs = setup.tile([P, 512], BF16)
            nc.vector.memset(warm_rhs, 0.001)
            warm_ps = sps_pool.tile([P, 512], FP32, tag="sps")
            for _ in range(20):
                nc.tensor.matmul(
                    warm_ps,
                    lhsT=warm_w,
                    rhs=warm_rhs,
                    start=True,
                    stop=True,
                )

            ones_col = setup.tile([P, 1], FP32)
            nc.vector.memset(ones_col, 1.0)
            ones_row = setup.tile([1, P], FP32)
            nc.vector.memset(ones_row, 1.0)

            # ---------------- main loop ----------------
            e_t = [None] * B_PER_CORE
            acc_t = [None] * B_PER_CORE

            def emit_xts(g, split=False):
                b, gb = divmod(g, NGRAN)
                if split:
                    for cc in range(GC):
                        if xts_t[g * GC + cc] is not None:
                            continue
                        xts_c = xts_pool.tile([P, 2, CHUNK], BF16, tag=f"xts0{cc}")
                        nc.sync.dma_start(
                            out=xts_c,
                            in_=xT[b, :, gb * GRAN + cc * CHUNK :
                                   gb * GRAN + (cc + 1) * CHUNK].rearrange(
                                "(a p) t -> p a t", p=P
                            ),
                        )
                        xts_t[g * GC + cc] = xts_c
                else:
                    xts = xts_pool.tile([P, 2, GRAN], BF16)
                    nc.sync.dma_start(
                        out=xts,
                        in_=xT[b, :, gb * GRAN : (gb + 1) * GRAN].rearrange(
                            "(a p) t -> p a t", p=P
                        ),
                    )
                    for cc in range(GC):
                        xts_t[g * GC + cc] = xts[:, :, cc * CHUNK : (cc + 1) * CHUNK]

            def emit_xin(g):
                b, gb = divmod(g, NGRAN)
                # xin[p, cc, r, m] = x[b, gb*4096 + cc*1024 + p*8 + r, m]
                xin = xin_pool.tile([P, GC, NT, M], BF16)
                nc.sync.dma_start(
                    out=xin,
                    in_=x[b, gb * GRAN : (gb + 1) * GRAN, :].rearrange(
                        "(cc p r) m -> p cc r m", p=P, r=NT
                    ),
                )
                for cc in range(GC):
                    xin_t[g * GC + cc] = xin[:, cc]

            g_t = [None] * NCT

            def emit_fold(ct):
                # DVE fold of the two n-halves: g = h0*vm0 + h1*vm1
                # (per-partition scalars; the partition sum happens on PE).
                # Emitted one iteration before its score matmuls so the
                # ~1.8us DVE chain overlaps a full chunk of PE work.
                # Dense writes only: a transposed out AP drops the DVE to
                # a ~5x slower path (measured).
                hsb = hsb_t[ct]
                g0 = g_pool.tile([P, P, NT], BF16, tag="g0")
                nc.vector.tensor_scalar_mul(g0, hsb[:, 0], vmt_f[:, 0:1])
                g = g_pool.tile([P, P, NT], BF16, tag="gg")
                nc.vector.scalar_tensor_tensor(
                    g, hsb[:, 1], vmt_f[:, 1:2], g0, ALU.mult, ALU.add
                )
                g_t[ct] = g
                hsb_t[ct] = None

            def emit_step(ct, sct, pct):
                """One pipeline step: h-matmuls for chunk ct interleaved
                with score matmuls for sct and pool matmuls for pct, so the
                512-col h streams cover the small matmuls' weight loads
                (one background weight slot => only loads issued during a
                long matmul are free)."""
                small = []
                if sct is not None:
                    b, c = divmod(sct, NCHUNK)
                    e_all = e_t[b]
                    g = g_t[sct]
                    sps = sps_pool.tile([P, NT], FP32, tag="sps")

                    def mk_sc(r):
                        def f():
                            # s[tau, r] = sum_p g[p, tau*8+r]
                            nc.tensor.matmul(
                                sps[:, r : r + 1],
                                lhsT=g[:, :, r],
                                rhs=ones_col_b,
                                start=True,
                                stop=True,
                            )
                        return f

                    small += [mk_sc(r) for r in range(NT)]
                if pct is not None:
                    bp, cp = divmod(pct, NCHUNK)
                    e_allp = e_t[bp]
                    acc_ps = acc_t[bp]
                    xin = xin_t[pct]

                    def mk_pool(r, mh):
                        def f():
                            # acc[q, mh] += sum_p x[t(p,r), mh*128+q]*e[t(p,r)]
                            nc.tensor.matmul(
                                acc_ps[:, mh : mh + 1],
                                lhsT=xin[:, r, mh * P : (mh + 1) * P],
                                rhs=e_allp[:, cp * NT + r : cp * NT + r + 1],
                                start=False,
                                stop=(cp == NCHUNK - 1 and r == NT - 1),
                                skip_group_check=True,
                            )
                        return f

                    small += [mk_pool(r, mh) for r in range(NT) for mh in range(2)]

                if ct is not None:
                    xts = xts_t[ct]
                    hsb = hsb_pool.tile([P, 2, P, NT], BF16)
                    hsb_t[ct] = hsb
                    si = 0
                    # h as 8x 512-col streams (512 = one PSUM bank: start
                    # clears has_written for the WHOLE bank, so start=True
                    # matmuls must cover full banks); 3 small matmuls after
                    # each h stream.  Finer splits measured slower.
                    for nh in range(2):
                        hps = hps_pool.tile([P, CHUNK], FP32)
                        for mh in range(2):
                            for hf in range(2):
                                nc.tensor.matmul(
                                    hps[:, hf * 512 : (hf + 1) * 512],
                                    lhsT=w1b[:, mh, nh * P : (nh + 1) * P],
                                    rhs=xts[:, mh, hf * 512 : (hf + 1) * 512],
                                    start=(mh == 0),
                                    stop=(mh == 1),
                                )
                                for _ in range(3):
                                    if si < len(small):
                                        small[si]()
                                        si += 1
                        nc.scalar.activation(
                            hsb[:, nh],
                            hps,
                            AF.Tanh,
                            bias=c_sb[:, nh : nh + 1],
                        )
                    while si < len(small):
                        small[si]()
                        si += 1
                else:
                    for f in small:
                        f()

                if sct is not None:
                    b, c = divmod(sct, NCHUNK)
                    nc.scalar.activation(
                        e_t[b][:, c * NT : (c + 1) * NT],
                        sps,
                        AF.Exp,
                    )
                if pct is not None:
                    xin_t[pct] = None

            def emit_finalize(b, use_gpsimd=False):
                e_all = e_t[b]
                acc_ps = acc_t[b]
                # Z = sum(e_all): free-dim reduce on DVE, then partition
                # reduce.  For example 0 (mid-pipeline) the partition reduce
                # runs on the idle GPSIMD so the in-order PE queue is not
                # blocked waiting on the DVE (which is busy with g-folds).
                z_red = fin_pool.tile([P, 1], FP32)
                nc.vector.reduce_sum(z_red, e_all, axis=mybir.AxisListType.X)
                rz = fin_pool.tile([P, 1], FP32, name=f"rz{b}")
                if use_gpsimd:
                    z_all = fin_pool.tile([P, 1], FP32, name="z_all")
                    nc.gpsimd.partition_all_reduce(
                        z_all, z_red, P, bass_isa.ReduceOp.add
                    )
                    nc.vector.reciprocal(rz, z_all)
                else:
                    z_ps = sps_pool.tile([1, 1], FP32, tag="sps")
                    nc.tensor.matmul(
                        z_ps, lhsT=z_red, rhs=ones_col, start=True, stop=True
                    )
                    z_sb = fin_pool.tile([1, 1], FP32)
                    nc.vector.tensor_copy(z_sb, z_ps)
                    # broadcast Z to all partitions, then reciprocal
                    zb_ps = sps_pool.tile([P, 1], FP32, tag="sps")
                    nc.tensor.matmul(
                        zb_ps, lhsT=ones_row, rhs=z_sb, start=True, stop=True
                    )
                    nc.vector.reciprocal(rz, zb_ps)
                # scale pooled sums by 1/Z; acc_ps is already m-partitioned
                outsb = fin_pool.tile([P, 2], FP32)
                nc.vector.tensor_scalar_mul(outsb, acc_ps, rz)
                nc.sync.dma_start(
                    out=out[b].rearrange("(a p) -> p a", p=P), in_=outsb
                )

            # both examples' accumulators share one PSUM bank ([P, 4]):
            # start=True would zero the whole bank row, wiping the sibling
            # chains; memset once and accumulate-only.
            acc_all = acc_pool.tile([P, 2 * B_PER_CORE], FP32, name="acc")
            nc.vector.memset(acc_all, 0.0)
            for b in range(B_PER_CORE):
                e_t[b] = e_pool.tile([P, NE], BF16, name=f"e_all{b}")
                acc_t[b] = acc_all[:, 2 * b : 2 * b + 2]

            # xts prefetch has priority over xin (pool trails h by 2 chunks)
            emit_xts(0, split=True)
            emit_xts(1)
            emit_xin(0)
            emit_xin(1)
            # scores lag h by 2 chunks, pool by 3: the per-chunk chain
            # (h-MM -> tanh -> DVE fold -> score-MM -> exp -> pool) is
            # ~4us, longer than one chunk of PE work, so a 1-chunk lag
            # stalls the PE waiting on the fold.
            for ct in range(NCT):
                # prefetch 2 granules ahead, issued early (ct%GC==2) so the
                # transfer completes well before the consuming chunk
                if ct % GC == 2 and ct // GC + 2 < NGT:
                    emit_xts(ct // GC + 2)
                    emit_xin(ct // GC + 2)
                if ct >= 1:
                    emit_fold(ct - 1)
                emit_step(
                    ct,
                    ct - 2 if ct >= 2 else None,
                    ct - 3 if ct >= 3 else None,
                )
                if ct == NCHUNK + 3:
                    emit_finalize(0, use_gpsimd=True)
            emit_fold(NCT - 1)
            emit_step(None, NCT - 2, NCT - 3)
            emit_step(None, NCT - 1, NCT - 2)
            emit_step(None, None, NCT - 1)
            emit_finalize(1)

    return nc


_PROGRAM_CACHE: list = []


def _get_program() -> bass.Bass:
    if not _PROGRAM_CACHE:
        nc = _build_program()
        nc.finalize()
        _PROGRAM_CACHE.append(nc)
    return _PROGRAM_CACHE[0]


def _make_in_maps(x, W1, W2, vm):
    xb = np.ascontiguousarray(x).astype(ml_dtypes.bfloat16)
    xbT = np.ascontiguousarray(xb.transpose(0, 2, 1))
    W1 = np.asarray(W1, dtype=np.float32)
    W2 = np.asarray(W2, dtype=np.float32)
    vm = np.asarray(vm, dtype=np.float32)
    # W1b[p, mh*256+n] = W1[mh*128+p, n]
    W1b = np.ascontiguousarray(
        W1.reshape(2, P, M).transpose(1, 0, 2).reshape(P, 2 * M)
    ).astype(ml_dtypes.bfloat16)
    c = (vm @ W2)[0]  # (256,)
    CV = np.ascontiguousarray(
        np.stack(
            [vm[0, :P], vm[0, P:], c[:P], c[P:]], axis=1
        ),
        dtype=np.float32,
    )
    return [
        {
            "x": xb[i * B_PER_CORE : (i + 1) * B_PER_CORE],
            "xT": xbT[i * B_PER_CORE : (i + 1) * B_PER_CORE],
            "W1b": W1b,
            "CV": CV,
        }
        for i in range(N_CORES)
    ]


def kernel(x, W1, W2, vm):
    nc = _get_program()
    in_maps = _make_in_maps(x, W1, W2, vm)
    res = run_bass_kernel_spmd(nc, in_maps, list(range(N_CORES)))
    out = np.concatenate([res.results[i]["out"] for i in range(N_CORES)], axis=0)
    return out.reshape(B, 1, M).astype(np.float32)
